# revision 49
# baseline (speedup 1.0000x reference)
"""Causal self-attention with RoPE on 8 Trainium2 NeuronCores.

Sharding: core c handles batch b = c//2 and head-group g = c%2 (8 of the 16
heads).  Wq/Wk/Wv are column-sharded (per head group), Wp is row-sharded;
each core computes a partial output projection for its batch and the host
sums the two partials per batch (the row-parallel unshard).

Device layouts (per core):
  xT    [C=1024, T=2048]  x transposed (contraction-friendly)
  wqT/wkT/wvT [1024, 512] W shard transposed ([c, d_local])
  wpT   [512, 1024]       Wp shard transposed ([c_local, e])
  cosT/sinT [128, 2048]   RoPE tables in [d, t] layout (2 head replicas,
                          sign folded into sinT for the rotate-half term)
  out   [2048, 1024] f32  partial projection output

Inside: q^T,k^T computed in [d, t] layout, v in [t, d]; scores computed
transposed (S^T = [k, t_q]) so softmax-normalizer and attention-output both
come from plain matmuls (V gets an appended ones-column to produce the
softmax denominator for free); causal mask applied post-exp via
affine_select (exact zeros).  All matmuls bf16 with fp32 PSUM accumulate.
"""

import os
import sys

sys.path.insert(0, "/opt/trn_rl_repo")

import ml_dtypes
import numpy as np

import concourse.bass as bass
import concourse.mybir as mybir
import concourse.tile as tile
from concourse import bacc
from concourse.bass_utils import run_bass_kernel_spmd

BF = mybir.dt.bfloat16
F32 = mybir.dt.float32
NPBF = ml_dtypes.bfloat16

B, T, C = 4, 2048, 1024
H, D = 16, 64
HL, DL = 8, 512  # heads / channels per core
NCT = C // 128  # 8 contraction tiles
NTT = T // 512  # 4 big time windows
NT16 = T // 128  # 16 small time windows
ROPE_BASE = 10000.0

SWAP_MASK = [i ^ 1 for i in range(32)]


def _build_nc():
    nc = bacc.Bacc("TRN2", target_bir_lowering=False, debug=False)

    xT_d = nc.dram_tensor("xT", [C, T], BF, kind="ExternalInput")
    wq_d = nc.dram_tensor("wqT", [C, DL], BF, kind="ExternalInput")
    wk_d = nc.dram_tensor("wkT", [C, DL], BF, kind="ExternalInput")
    wv_d = nc.dram_tensor("wvT", [C, DL], BF, kind="ExternalInput")
    wp_d = nc.dram_tensor("wpT", [DL, C], BF, kind="ExternalInput")
    cos_d = nc.dram_tensor("cosT", [128, T], BF, kind="ExternalInput")
    sin_d = nc.dram_tensor("sinT", [128, T], BF, kind="ExternalInput")
    out_d = nc.dram_tensor("out", [T, C], F32, kind="ExternalOutput")

    with tile.TileContext(nc) as tc:
        _body(nc, tc, xT_d, wq_d, wk_d, wv_d, wp_d, cos_d, sin_d, out_d)
    nc.compile()
    return nc


def _body(nc, tc, xT_d, wq_d, wk_d, wv_d, wp_d, cos_d, sin_d, out_d):
    import contextlib

    ctx = contextlib.ExitStack()
    with ctx:
        const = ctx.enter_context(tc.tile_pool(name="const", bufs=1))
        work = ctx.enter_context(tc.tile_pool(name="work", bufs=2))
        psum = ctx.enter_context(tc.tile_pool(name="psum", bufs=1, space="PSUM"))

        # ---- resident SBUF tensors -------------------------------------
        # inputs are loaded per contraction-tile in the order the first
        # projection matmuls consume them, so the PE starts within a few us
        x_sb = const.tile([128, NCT, T], BF)
        wq_sb = const.tile([128, NCT, DL], BF)
        wk_sb = const.tile([128, NCT, DL], BF)
        wv_sb = const.tile([128, NCT, DL], BF)
        x_dr = xT_d[:].rearrange("(a p) t -> p a t", p=128)
        wq_dr = wq_d[:].rearrange("(a p) d -> p a d", p=128)
        wk_dr = wk_d[:].rearrange("(a p) d -> p a d", p=128)
        wv_dr = wv_d[:].rearrange("(a p) d -> p a d", p=128)
        # startup is DMA-descriptor-issue-bound (~0.7us per dma_start on a
        # queue), so batch multi-tile loads into single descriptors and split
        # them across the two HWDGE queues (Sync: x / rope tables, Scalar:
        # weights + later x windows).  x[ct0, w0] gets its own tiny leading
        # descriptor so the first matmul group isn't gated on the full 1MB.
        w0 = slice(0, 512)
        nc.sync.dma_start(out=x_sb[:, 0, w0], in_=x_dr[:, 0, w0])
        nc.sync.dma_start(out=x_sb[:, 1:NCT, w0], in_=x_dr[:, 1:NCT, w0])
        # piece(0,0) only consumes the m=0 slice of wq/wk — land it first
        nc.scalar.dma_start(out=wq_sb[:, :, 0:128], in_=wq_dr[:, :, 0:128])
        nc.scalar.dma_start(out=wk_sb[:, :, 0:128], in_=wk_dr[:, :, 0:128])
        nc.scalar.dma_start(out=wq_sb[:, :, 128:512], in_=wq_dr[:, :, 128:512])
        nc.scalar.dma_start(out=wk_sb[:, :, 128:512], in_=wk_dr[:, :, 128:512])
        cos_sb = const.tile([128, T], BF)
        nc.sync.dma_start(out=cos_sb, in_=cos_d[:])
        sin_sb = const.tile([128, T], BF)
        nc.sync.dma_start(out=sin_sb, in_=sin_d[:])
        nc.scalar.dma_start(out=wv_sb, in_=wv_dr)
        for wt in range(1, NTT):
            wsl = slice(wt * 512, (wt + 1) * 512)
            nc.sync.dma_start(out=x_sb[:, :, wsl], in_=x_dr[:, :, wsl])
        wp_sb = const.tile([128, 4, C], BF)
        nc.scalar.dma_start(
            out=wp_sb, in_=wp_d[:].rearrange("(a p) e -> p a e", p=128)
        )

        # v in [t, h, d(+ones)] layout; col 64 of each head group is 1.0
        v_sb = const.tile([128, NT16, HL, 65], BF)
        nc.vector.memset(v_sb[:, :, :, 64], 1.0)

        # causal mask for diagonal score tiles: tri[k, j] = (j >= k).  Every
        # diagonal k-tile reduces to this same pattern once sliced at its q0,
        # so one [128, 512] constant serves all of them (applied as a DVE
        # multiply, keeping the gpsimd FIFO free for the epilogue broadcasts)
        tri_sb = const.tile([128, 512], BF)
        nc.vector.memset(tri_sb, 1.0)
        nc.gpsimd.affine_select(
            tri_sb,
            tri_sb,
            pattern=[[1, 512]],
            compare_op=mybir.AluOpType.is_ge,
            fill=0.0,
            base=0,
            channel_multiplier=-1,
        )

        qr_sb = const.tile([128, 4, T], BF)  # q^T after rope, 4 head-pair tiles
        kr_sb = const.tile([128, 4, T], BF)
        yT_sb = const.tile([128, 4, T], BF)  # attention out, pre-projection

        # ---- per-window phase bodies -----------------------------------
        def rope_evac(ps, tsl, nm):
            # DVE copy: ACT is the second-busiest engine (softmax exps); DVE
            # has slack once the reciprocals are batched/approximated
            ev = work.tile([128, 512], BF, tag="ev", bufs=2, name=f"ev{nm}")
            nc.vector.tensor_copy(ev, ps)
            sh = work.tile([128, 512], BF, tag="sh", bufs=2, name=f"sh{nm}")
            nc.vector.stream_shuffle(sh, ev, SWAP_MASK)
            t1 = work.tile([128, 512], BF, tag="t1", bufs=3, name=f"t1{nm}")
            nc.vector.tensor_mul(t1, sh, sin_sb[:, tsl])
            t2 = work.tile([128, 512], BF, tag="t2", bufs=3, name=f"t2{nm}")
            nc.vector.tensor_mul(t2, ev, cos_sb[:, tsl])
            return t1, t2

        def proj_qkv_piece(tt, m):
            """Generator: q^T,k^T (+rope) for pair m and v for t16=4tt+m of
            time window tt.  Yields between matmuls so the caller can
            interleave these as PE filler inside attention."""
            tsl = slice(tt * 512, (tt + 1) * 512)
            dsl = slice(m * 128, (m + 1) * 128)
            for W, dst, nm in ((wq_sb, qr_sb, "q"), (wk_sb, kr_sb, "k")):
                ps = psum.tile([128, 512], F32, tag="pj", bufs=2, name=f"{nm}p{tt}_{m}")
                for ct in range(NCT):
                    nc.tensor.matmul(
                        ps,
                        lhsT=W[:, ct, dsl],
                        rhs=x_sb[:, ct, tsl],
                        start=(ct == 0),
                        stop=(ct == NCT - 1),
                    )
                    yield
                t1, t2 = rope_evac(ps, tsl, f"{nm}{tt}_{m}")
                nc.vector.tensor_add(dst[:, m, tsl], t1, t2)
                yield
            t16 = 4 * tt + m
            vp = psum.tile([128, 512], F32, tag="pj", bufs=2, name=f"vp{t16}")
            for ct in range(NCT):
                nc.tensor.matmul(
                    vp,
                    lhsT=x_sb[:, ct, t16 * 128 : (t16 + 1) * 128],
                    rhs=wv_sb[:, ct, :],
                    start=(ct == 0),
                    stop=(ct == NCT - 1),
                )
                yield
            nc.scalar.copy(
                v_sb[:, t16, :, 0:64], vp.rearrange("p (h d) -> p h d", h=HL)
            )
            yield

        def emit_st(m, qt, ki):
            # one [128,1024] tile: head A scores in cols 0-511 (bank 1),
            # head B in cols 512-1023 (bank 2); the two matmuls run
            # concurrently in disjoint PE row groups (K=64 each).
            # Diagonal k-tiles only compute the live (unmasked) q-range
            # [q0, 512) — q columns below 128*(ki-4qt) are fully masked.
            #
            # The whole S^T -> exp -> mask chain runs at high priority: the
            # exp stream is the kernel's scarcest resource, and without the
            # boost the static scheduler happily runs a dozen ready filler
            # matmuls between S^T tiles, pacing the exps down with them.
            q0 = max(0, 128 * ki - 512 * qt)
            w = 512 - q0
            st = psum.tile([128, 1024], F32, tag="st", bufs=2, name=f"st{m}_{qt}_{ki}")
            with tc.high_priority(offset=400):
                for h2 in (0, 1):
                    rsl = slice(64 * h2, 64 * h2 + 64)
                    nc.tensor.matmul(
                        st[:, h2 * 512 + q0 : (h2 + 1) * 512],
                        lhsT=kr_sb[rsl, m, ki * 128 : (ki + 1) * 128],
                        rhs=qr_sb[rsl, m, qt * 512 + q0 : (qt + 1) * 512],
                        start=True,
                        stop=True,
                    )
                pt = work.tile([128, 1024], BF, tag="pt", bufs=9, name=f"pt{m}_{qt}_{ki}")
                stv = st.rearrange("p (g c) -> p g c", g=2)[:, :, q0:512]
                ptv = pt.rearrange("p (g c) -> p g c", g=2)[:, :, q0:512]
                nc.scalar.activation(
                    ptv, stv, mybir.ActivationFunctionType.Exp, scale=0.125
                )
                if ki >= 4 * qt:  # diagonal block: causal mask (both halves)
                    for h2 in (0, 1):
                        msl = pt[:, h2 * 512 + q0 : (h2 + 1) * 512]
                        nc.vector.tensor_mul(msl, msl, tri_sb[:, 0:w])
            return pt

        def finish_pair(m, qt, yxs):
            # evacuate PSUM immediately for BOTH heads (frees the yx banks
            # fast); the slow division chain is deferred — its result is
            # first needed by the output projection much later.
            qsl = slice(qt * 512, (qt + 1) * 512)
            ysrs = []
            for h2 in (0, 1):
                ysr = work.tile(
                    [65, 512], F32, tag="ysr", bufs=8, name=f"ysr{m}_{qt}_{h2}"
                )
                nc.vector.tensor_copy(ysr, yxs[h2])
                ysrs.append(ysr)

            def epilogue():
                # everything except the recip (a custom-DVE-only op) and the
                # psum evacs runs on GPSIMD: the DVE queue is what delays the
                # next window's rope chains at window boundaries, while
                # GPSIMD is mostly idle
                rls = []
                for h2 in (0, 1):
                    # the custom-DVE approx only works at partition base 0, so
                    # stage the denominator row (partition 64) down first
                    dnc = work.tile(
                        [1, 512], F32, tag="dnc", bufs=2, name=f"dn{m}_{qt}_{h2}"
                    )
                    nc.vector.tensor_copy(dnc, ysrs[h2][64:65, :])
                    rl = work.tile(
                        [1, 512], F32, tag="rl", bufs=2, name=f"rl{m}_{qt}_{h2}"
                    )
                    nc.vector.reciprocal_approx_fast(rl, dnc)
                    rls.append(rl)
                for h2 in (0, 1):
                    rsl = slice(64 * h2, 64 * h2 + 64)
                    rlb = work.tile(
                        [64, 512], F32, tag="rlb", bufs=2, name=f"rlb{m}_{qt}_{h2}"
                    )
                    nc.gpsimd.partition_broadcast(rlb, rls[h2])
                    nc.vector.tensor_mul(yT_sb[rsl, m, qsl], ysrs[h2][0:64, :], rlb)

            return epilogue

        # ---- the global attention pipeline -----------------------------
        # one flat (qt, m, ki) stream: the lag-4 AV emission crosses pair
        # AND window boundaries, so the S^T/exp stream (the scarce ACT
        # resource) never drains serially at a boundary.  yx psum rotation
        # stays legal because a pair's evacuation is emitted before the
        # next-but-one pair's first AV needs the buffers.
        pend = []       # (qt, m, ki, pt) awaiting their AV matmuls
        yxs_by = {}     # (qt, m) -> yx psum tiles
        eps_carry = []  # deferred division chains

        def consume(last=False):
            dqt, dm, dki, dpt = pend.pop(0)
            dnk = 4 * dqt + 4
            q0 = max(0, 128 * dki - 512 * dqt)
            key = (dqt, dm)
            if key not in yxs_by:
                yxs_by[key] = [
                    psum.tile(
                        [65, 512], F32, tag="yx", bufs=2,
                        name=f"yx{dm}_{dqt}_{h2}",
                    )
                    for h2 in (0, 1)
                ]
            yxs = yxs_by[key]
            with tc.high_priority(offset=200):
                for h2 in (0, 1):
                    nc.tensor.matmul(
                        yxs[h2][:, q0:512],
                        lhsT=v_sb[:, dki, 2 * dm + h2, :],
                        rhs=dpt[:, h2 * 512 + q0 : (h2 + 1) * 512],
                        start=(dki == 0),
                        stop=(dki == dnk - 1),
                    )
            if dki == dnk - 1:
                eps_carry.append(finish_pair(dm, dqt, yxs_by.pop(key)))
                # run division chains one finished pair behind (immediately
                # in the last window so the tail projection isn't gated)
                if last or len(eps_carry) > 1:
                    eps_carry.pop(0)()

        def window(qt, filler, nfill, last=False):
            nk = 4 * qt + 4
            seq = [(m, ki) for m in range(4) for ki in range(nk)]
            n = len(seq)
            taken = 0
            for idx, (m, ki) in enumerate(seq):
                pend.append((qt, m, ki, emit_st(m, qt, ki)))
                # spread the filler budget evenly across the window's tiles:
                # a front-loaded fixed rate exhausts the filler early and
                # leaves the tail k-tiles exp-paced (PE idle)
                want = (idx + 1) * nfill // n
                while taken < want:
                    next(filler, None)
                    taken += 1
                if len(pend) > 4:
                    consume(last)
                # division chains run spread across the tile stream, away
                # from pair finishes and window boundaries
                if eps_carry and idx % 4 == 3:
                    eps_carry.pop(0)()
            if last:
                while pend:
                    consume(True)
                while eps_carry:
                    eps_carry.pop(0)()

        def proj_out_piece(t16):
            """Generator: output-projection partial for time window t16."""
            t16sl = slice(t16 * 128, (t16 + 1) * 128)
            osb = work.tile([128, C], F32, tag="osb", bufs=3, name=f"osb{t16}")
            for e2 in (0, 1):
                op = psum.tile([128, 512], F32, tag="pj", bufs=2, name=f"op{t16}_{e2}")
                for hdt in range(4):
                    nc.tensor.matmul(
                        op,
                        lhsT=yT_sb[:, hdt, t16sl],
                        rhs=wp_sb[:, hdt, e2 * 512 : (e2 + 1) * 512],
                        start=(hdt == 0),
                        stop=(hdt == 3),
                    )
                    yield
                nc.vector.tensor_copy(osb[:, e2 * 512 : (e2 + 1) * 512], op)
                yield
            nc.sync.dma_start(out=out_d[t16sl, :], in_=osb)
            yield

        def chain(*gens):
            for g in gens:
                yield from g

        def drain(g):
            for _ in g:
                pass

        # ---- merged schedule ------------------------------------------
        # attention at query-window qt needs q/k/v only through window qt.
        # Only pair 0's projections run up front (getting the scarce exp
        # stream started early); the rest of window 0, window tt+1's
        # projections, and output-projection pieces are fed as PE filler
        # INSIDE the flattened attention windows, so the PE always has
        # independent matmuls while ACT streams the softmax exps.  Window
        # 0's filler leads with the remaining window-0 pieces: its own
        # pairs need their q/k ropes and v(t16 0-3).
        drain(proj_qkv_piece(0, 0))
        for tt in range(NTT):
            gens = []
            nfill = 0
            if tt == 0:
                for pc in ((0, 1), (0, 2), (0, 3), (1, 0), (1, 1), (1, 2), (1, 3)):
                    gens.append(proj_qkv_piece(*pc))
                    nfill += 27
            elif tt + 1 < NTT:
                for m in range(4):
                    gens.append(proj_qkv_piece(tt + 1, m))
                    nfill += 27
            if tt == 2:
                for t16 in range(4):
                    gens.append(proj_out_piece(t16))
                    nfill += 11
            elif tt == 3:
                for t16 in range(4, 12):
                    gens.append(proj_out_piece(t16))
                    nfill += 11
            filler = chain(*gens)
            window(tt, filler, nfill, last=(tt == NTT - 1))
            drain(filler)
        for m in range(4):
            drain(proj_out_piece(12 + m))


_NC_CACHE = None
LAST_RESULT = None


def _get_nc():
    global _NC_CACHE
    if _NC_CACHE is None:
        _NC_CACHE = _build_nc()
    return _NC_CACHE


def _rope_tables(start_pos):
    inv = 1.0 / (ROPE_BASE ** (np.arange(0, D, 2, dtype=np.float32) / D))
    t = np.arange(T, dtype=np.float32) + np.float32(start_pos)
    freqs = t[:, None] * inv[None, :]  # [T, 32]
    emb = np.concatenate([freqs, freqs], axis=-1)  # [T, 64]
    cos = np.cos(emb).T  # [64, T]
    sin = np.sin(emb).T
    sgn = np.where(np.arange(D) % 2 == 0, -1.0, 1.0).astype(np.float32)
    cosT = np.tile(cos, (2, 1))
    sinT = np.tile(sin * sgn[:, None], (2, 1))
    return cosT.astype(NPBF), sinT.astype(NPBF)


def kernel(x, Wq, Wk, Wv, Wp, start_pos):
    x = np.asarray(x, dtype=np.float32)
    Wq = np.asarray(Wq, dtype=np.float32)
    Wk = np.asarray(Wk, dtype=np.float32)
    Wv = np.asarray(Wv, dtype=np.float32)
    Wp = np.asarray(Wp, dtype=np.float32)
    cosT, sinT = _rope_tables(int(start_pos))

    nc = _get_nc()
    in_maps = []
    for c in range(8):
        b, g = divmod(c, 2)
        hs = slice(g * DL, (g + 1) * DL)
        in_maps.append(
            {
                "xT": np.ascontiguousarray(x[b].T).astype(NPBF),
                "wqT": np.ascontiguousarray(Wq[hs, :].T).astype(NPBF),
                "wkT": np.ascontiguousarray(Wk[hs, :].T).astype(NPBF),
                "wvT": np.ascontiguousarray(Wv[hs, :].T).astype(NPBF),
                "wpT": np.ascontiguousarray(Wp[:, hs].T).astype(NPBF),
                "cosT": cosT,
                "sinT": sinT,
            }
        )
    try:
        res = run_bass_kernel_spmd(nc, in_maps, core_ids=list(range(8)))
    except ModuleNotFoundError:
        # BASS_TRACE set but the axon NTFF hook module is unavailable in
        # this environment — rerun with tracing disabled.
        os.environ["BASS_NEVER_TRACE"] = "1"
        res = run_bass_kernel_spmd(nc, in_maps, core_ids=list(range(8)))
    global LAST_RESULT
    LAST_RESULT = res
    outs = [r["out"] for r in res.results]
    full = np.stack(
        [outs[2 * b] + outs[2 * b + 1] for b in range(B)], axis=0
    )
    return full.astype(np.float32)


if __name__ == "__main__":
    nc = _get_nc()
    print("built ok:", len(nc.m.functions[0].blocks if hasattr(nc.m.functions[0], 'blocks') else []), "blocks")



# revision 50
# speedup vs baseline: 1.0088x; 1.0088x over previous
"""Causal self-attention with RoPE on 8 Trainium2 NeuronCores.

Sharding: core c handles batch b = c//2 and head-group g = c%2 (8 of the 16
heads).  Wq/Wk/Wv are column-sharded (per head group), Wp is row-sharded;
each core computes a partial output projection for its batch and the host
sums the two partials per batch (the row-parallel unshard).

Device layouts (per core):
  xT    [C=1024, T=2048]  x transposed (contraction-friendly)
  wqT/wkT/wvT [1024, 512] W shard transposed ([c, d_local])
  wpT   [512, 1024]       Wp shard transposed ([c_local, e])
  cosT/sinT [128, 2048]   RoPE tables in [d, t] layout (2 head replicas,
                          sign folded into sinT for the rotate-half term)
  out   [2048, 1024] f32  partial projection output

Inside: q^T,k^T computed in [d, t] layout, v in [t, d]; scores computed
transposed (S^T = [k, t_q]) so softmax-normalizer and attention-output both
come from plain matmuls (V gets an appended ones-column to produce the
softmax denominator for free); causal mask applied post-exp via
affine_select (exact zeros).  All matmuls bf16 with fp32 PSUM accumulate.
"""

import os
import sys

sys.path.insert(0, "/opt/trn_rl_repo")

import ml_dtypes
import numpy as np

import concourse.bass as bass
import concourse.mybir as mybir
import concourse.tile as tile
from concourse import bacc
from concourse.bass_utils import run_bass_kernel_spmd

BF = mybir.dt.bfloat16
F32 = mybir.dt.float32
NPBF = ml_dtypes.bfloat16

B, T, C = 4, 2048, 1024
H, D = 16, 64
HL, DL = 8, 512  # heads / channels per core
NCT = C // 128  # 8 contraction tiles
NTT = T // 512  # 4 big time windows
NT16 = T // 128  # 16 small time windows
ROPE_BASE = 10000.0

SWAP_MASK = [i ^ 1 for i in range(32)]


def _build_nc():
    nc = bacc.Bacc("TRN2", target_bir_lowering=False, debug=False)

    xT_d = nc.dram_tensor("xT", [C, T], BF, kind="ExternalInput")
    wq_d = nc.dram_tensor("wqT", [C, DL], BF, kind="ExternalInput")
    wk_d = nc.dram_tensor("wkT", [C, DL], BF, kind="ExternalInput")
    wv_d = nc.dram_tensor("wvT", [C, DL], BF, kind="ExternalInput")
    wp_d = nc.dram_tensor("wpT", [DL, C], BF, kind="ExternalInput")
    cos_d = nc.dram_tensor("cosT", [128, T], BF, kind="ExternalInput")
    sin_d = nc.dram_tensor("sinT", [128, T], BF, kind="ExternalInput")
    out_d = nc.dram_tensor("out", [T, C], F32, kind="ExternalOutput")

    with tile.TileContext(nc) as tc:
        _body(nc, tc, xT_d, wq_d, wk_d, wv_d, wp_d, cos_d, sin_d, out_d)
    nc.compile()
    return nc


def _body(nc, tc, xT_d, wq_d, wk_d, wv_d, wp_d, cos_d, sin_d, out_d):
    import contextlib

    ctx = contextlib.ExitStack()
    with ctx:
        const = ctx.enter_context(tc.tile_pool(name="const", bufs=1))
        work = ctx.enter_context(tc.tile_pool(name="work", bufs=2))
        psum = ctx.enter_context(tc.tile_pool(name="psum", bufs=1, space="PSUM"))

        # ---- resident SBUF tensors -------------------------------------
        # inputs are loaded per contraction-tile in the order the first
        # projection matmuls consume them, so the PE starts within a few us
        x_sb = const.tile([128, NCT, T], BF)
        wq_sb = const.tile([128, NCT, DL], BF)
        wk_sb = const.tile([128, NCT, DL], BF)
        wv_sb = const.tile([128, NCT, DL], BF)
        x_dr = xT_d[:].rearrange("(a p) t -> p a t", p=128)
        wq_dr = wq_d[:].rearrange("(a p) d -> p a d", p=128)
        wk_dr = wk_d[:].rearrange("(a p) d -> p a d", p=128)
        wv_dr = wv_d[:].rearrange("(a p) d -> p a d", p=128)
        # startup is DMA-descriptor-issue-bound (~0.7us per dma_start on a
        # queue), so batch multi-tile loads into single descriptors and split
        # them across the two HWDGE queues (Sync: x / rope tables, Scalar:
        # weights + later x windows).  x[ct0, w0] gets its own tiny leading
        # descriptor so the first matmul group isn't gated on the full 1MB.
        w0 = slice(0, 512)
        nc.sync.dma_start(out=x_sb[:, 0, w0], in_=x_dr[:, 0, w0])
        nc.sync.dma_start(out=x_sb[:, 1:NCT, w0], in_=x_dr[:, 1:NCT, w0])
        # piece(0,0) only consumes the m=0 slice of wq/wk — land it first
        nc.scalar.dma_start(out=wq_sb[:, :, 0:128], in_=wq_dr[:, :, 0:128])
        nc.scalar.dma_start(out=wk_sb[:, :, 0:128], in_=wk_dr[:, :, 0:128])
        nc.scalar.dma_start(out=wq_sb[:, :, 128:512], in_=wq_dr[:, :, 128:512])
        nc.scalar.dma_start(out=wk_sb[:, :, 128:512], in_=wk_dr[:, :, 128:512])
        cos_sb = const.tile([128, T], BF)
        nc.sync.dma_start(out=cos_sb, in_=cos_d[:])
        sin_sb = const.tile([128, T], BF)
        nc.sync.dma_start(out=sin_sb, in_=sin_d[:])
        nc.scalar.dma_start(out=wv_sb, in_=wv_dr)
        for wt in range(1, NTT):
            wsl = slice(wt * 512, (wt + 1) * 512)
            nc.sync.dma_start(out=x_sb[:, :, wsl], in_=x_dr[:, :, wsl])
        wp_sb = const.tile([128, 4, C], BF)
        nc.scalar.dma_start(
            out=wp_sb, in_=wp_d[:].rearrange("(a p) e -> p a e", p=128)
        )

        # v in [t, h, d(+ones)] layout; col 64 of each head group is 1.0
        v_sb = const.tile([128, NT16, HL, 65], BF)
        nc.vector.memset(v_sb[:, :, :, 64], 1.0)

        # causal mask for diagonal score tiles: tri[k, j] = (j >= k).  Every
        # diagonal k-tile reduces to this same pattern once sliced at its q0,
        # so one [128, 512] constant serves all of them (applied as a DVE
        # multiply, keeping the gpsimd FIFO free for the epilogue broadcasts)
        tri_sb = const.tile([128, 512], BF)
        nc.vector.memset(tri_sb, 1.0)
        nc.gpsimd.affine_select(
            tri_sb,
            tri_sb,
            pattern=[[1, 512]],
            compare_op=mybir.AluOpType.is_ge,
            fill=0.0,
            base=0,
            channel_multiplier=-1,
        )

        qr_sb = const.tile([128, 4, T], BF)  # q^T after rope, 4 head-pair tiles
        kr_sb = const.tile([128, 4, T], BF)
        yT_sb = const.tile([128, 4, T], BF)  # attention out, pre-projection

        # ---- per-window phase bodies -----------------------------------
        def rope_evac(ps, tsl, nm):
            # DVE copy: ACT is the second-busiest engine (softmax exps); DVE
            # has slack once the reciprocals are batched/approximated
            ev = work.tile([128, 512], BF, tag="ev", bufs=2, name=f"ev{nm}")
            nc.vector.tensor_copy(ev, ps)
            sh = work.tile([128, 512], BF, tag="sh", bufs=2, name=f"sh{nm}")
            nc.vector.stream_shuffle(sh, ev, SWAP_MASK)
            t1 = work.tile([128, 512], BF, tag="t1", bufs=3, name=f"t1{nm}")
            nc.vector.tensor_mul(t1, sh, sin_sb[:, tsl])
            t2 = work.tile([128, 512], BF, tag="t2", bufs=3, name=f"t2{nm}")
            nc.vector.tensor_mul(t2, ev, cos_sb[:, tsl])
            return t1, t2

        def proj_qkv_piece(tt, m):
            """Generator: q^T,k^T (+rope) for pair m and v for t16=4tt+m of
            time window tt.  Yields between matmuls so the caller can
            interleave these as PE filler inside attention."""
            tsl = slice(tt * 512, (tt + 1) * 512)
            dsl = slice(m * 128, (m + 1) * 128)
            for W, dst, nm in ((wq_sb, qr_sb, "q"), (wk_sb, kr_sb, "k")):
                ps = psum.tile([128, 512], F32, tag="pj", bufs=2, name=f"{nm}p{tt}_{m}")
                for ct in range(NCT):
                    nc.tensor.matmul(
                        ps,
                        lhsT=W[:, ct, dsl],
                        rhs=x_sb[:, ct, tsl],
                        start=(ct == 0),
                        stop=(ct == NCT - 1),
                    )
                    yield
                t1, t2 = rope_evac(ps, tsl, f"{nm}{tt}_{m}")
                nc.vector.tensor_add(dst[:, m, tsl], t1, t2)
                yield
            t16 = 4 * tt + m
            vp = psum.tile([128, 512], F32, tag="pj", bufs=2, name=f"vp{t16}")
            for ct in range(NCT):
                nc.tensor.matmul(
                    vp,
                    lhsT=x_sb[:, ct, t16 * 128 : (t16 + 1) * 128],
                    rhs=wv_sb[:, ct, :],
                    start=(ct == 0),
                    stop=(ct == NCT - 1),
                )
                yield
            nc.scalar.copy(
                v_sb[:, t16, :, 0:64], vp.rearrange("p (h d) -> p h d", h=HL)
            )
            yield

        def emit_st(m, qt, ki):
            # one [128,1024] tile: head A scores in cols 0-511 (bank 1),
            # head B in cols 512-1023 (bank 2); the two matmuls run
            # concurrently in disjoint PE row groups (K=64 each).
            # Diagonal k-tiles only compute the live (unmasked) q-range
            # [q0, 512) — q columns below 128*(ki-4qt) are fully masked.
            #
            # The whole S^T -> exp -> mask chain runs at high priority: the
            # exp stream is the kernel's scarcest resource, and without the
            # boost the static scheduler happily runs a dozen ready filler
            # matmuls between S^T tiles, pacing the exps down with them.
            q0 = max(0, 128 * ki - 512 * qt)
            w = 512 - q0
            st = psum.tile([128, 1024], F32, tag="st", bufs=2, name=f"st{m}_{qt}_{ki}")
            with tc.high_priority(offset=400):
                for h2 in (0, 1):
                    rsl = slice(64 * h2, 64 * h2 + 64)
                    nc.tensor.matmul(
                        st[:, h2 * 512 + q0 : (h2 + 1) * 512],
                        lhsT=kr_sb[rsl, m, ki * 128 : (ki + 1) * 128],
                        rhs=qr_sb[rsl, m, qt * 512 + q0 : (qt + 1) * 512],
                        start=True,
                        stop=True,
                    )
                pt = work.tile([128, 1024], BF, tag="pt", bufs=8, name=f"pt{m}_{qt}_{ki}")
                stv = st.rearrange("p (g c) -> p g c", g=2)[:, :, q0:512]
                ptv = pt.rearrange("p (g c) -> p g c", g=2)[:, :, q0:512]
                nc.scalar.activation(
                    ptv, stv, mybir.ActivationFunctionType.Exp, scale=0.125
                )
                if ki >= 4 * qt:  # diagonal block: causal mask (both halves)
                    for h2 in (0, 1):
                        msl = pt[:, h2 * 512 + q0 : (h2 + 1) * 512]
                        nc.vector.tensor_mul(msl, msl, tri_sb[:, 0:w])
            return pt

        def finish_pair(m, qt, yxs):
            # evacuate PSUM immediately for BOTH heads (frees the yx banks
            # fast); the slow division chain is deferred — its result is
            # first needed by the output projection much later.
            qsl = slice(qt * 512, (qt + 1) * 512)
            ysrs = []
            for h2 in (0, 1):
                ysr = work.tile(
                    [65, 512], F32, tag="ysr", bufs=8, name=f"ysr{m}_{qt}_{h2}"
                )
                nc.vector.tensor_copy(ysr, yxs[h2])
                ysrs.append(ysr)

            def epilogue():
                # everything except the recip (a custom-DVE-only op) and the
                # psum evacs runs on GPSIMD: the DVE queue is what delays the
                # next window's rope chains at window boundaries, while
                # GPSIMD is mostly idle
                rls = []
                for h2 in (0, 1):
                    # the custom-DVE approx only works at partition base 0, so
                    # stage the denominator row (partition 64) down first
                    dnc = work.tile(
                        [1, 512], F32, tag="dnc", bufs=2, name=f"dn{m}_{qt}_{h2}"
                    )
                    nc.vector.tensor_copy(dnc, ysrs[h2][64:65, :])
                    rl = work.tile(
                        [1, 512], F32, tag="rl", bufs=2, name=f"rl{m}_{qt}_{h2}"
                    )
                    nc.vector.reciprocal_approx_fast(rl, dnc)
                    rls.append(rl)
                for h2 in (0, 1):
                    rsl = slice(64 * h2, 64 * h2 + 64)
                    rlb = work.tile(
                        [64, 512], F32, tag="rlb", bufs=2, name=f"rlb{m}_{qt}_{h2}"
                    )
                    nc.gpsimd.partition_broadcast(rlb, rls[h2])
                    nc.vector.tensor_mul(yT_sb[rsl, m, qsl], ysrs[h2][0:64, :], rlb)

            return epilogue

        # ---- the global attention pipeline -----------------------------
        # one flat (qt, m, ki) stream: the lag-4 AV emission crosses pair
        # AND window boundaries, so the S^T/exp stream (the scarce ACT
        # resource) never drains serially at a boundary.  yx psum rotation
        # stays legal because a pair's evacuation is emitted before the
        # next-but-one pair's first AV needs the buffers.
        pend = []       # (qt, m, ki, pt) awaiting their AV matmuls
        yxs_by = {}     # (qt, m) -> yx psum tiles
        eps_carry = []  # deferred division chains

        def consume(last=False):
            dqt, dm, dki, dpt = pend.pop(0)
            dnk = 4 * dqt + 4
            q0 = max(0, 128 * dki - 512 * dqt)
            key = (dqt, dm)
            if key not in yxs_by:
                yxs_by[key] = [
                    psum.tile(
                        [65, 512], F32, tag="yx", bufs=2,
                        name=f"yx{dm}_{dqt}_{h2}",
                    )
                    for h2 in (0, 1)
                ]
            yxs = yxs_by[key]
            with tc.high_priority(offset=200):
                for h2 in (0, 1):
                    nc.tensor.matmul(
                        yxs[h2][:, q0:512],
                        lhsT=v_sb[:, dki, 2 * dm + h2, :],
                        rhs=dpt[:, h2 * 512 + q0 : (h2 + 1) * 512],
                        start=(dki == 0),
                        stop=(dki == dnk - 1),
                    )
            if dki == dnk - 1:
                eps_carry.append(finish_pair(dm, dqt, yxs_by.pop(key)))
                # run division chains one finished pair behind (immediately
                # in the last window so the tail projection isn't gated)
                if last or len(eps_carry) > 1:
                    eps_carry.pop(0)()

        def window(qt, filler, nfill, last=False):
            nk = 4 * qt + 4
            seq = [(m, ki) for m in range(4) for ki in range(nk)]
            n = len(seq)
            taken = 0
            for idx, (m, ki) in enumerate(seq):
                pend.append((qt, m, ki, emit_st(m, qt, ki)))
                # spread the filler budget evenly across the window's tiles:
                # a front-loaded fixed rate exhausts the filler early and
                # leaves the tail k-tiles exp-paced (PE idle)
                want = (idx + 1) * nfill // n
                while taken < want:
                    next(filler, None)
                    taken += 1
                if len(pend) > 4:
                    consume(last)
                # division chains run spread across the tile stream, away
                # from pair finishes and window boundaries
                if eps_carry and idx % 4 == 3:
                    eps_carry.pop(0)()
            # drain this window's AV tail (pt buffers are too tight to carry
            # four score tiles into the next window's S^T stream)
            while pend:
                consume(last)
            if last:
                while eps_carry:
                    eps_carry.pop(0)()

        def proj_out_piece(t16):
            """Generator: output-projection partial for time window t16."""
            t16sl = slice(t16 * 128, (t16 + 1) * 128)
            osb = work.tile([128, C], F32, tag="osb", bufs=3, name=f"osb{t16}")
            for e2 in (0, 1):
                op = psum.tile([128, 512], F32, tag="pj", bufs=2, name=f"op{t16}_{e2}")
                for hdt in range(4):
                    nc.tensor.matmul(
                        op,
                        lhsT=yT_sb[:, hdt, t16sl],
                        rhs=wp_sb[:, hdt, e2 * 512 : (e2 + 1) * 512],
                        start=(hdt == 0),
                        stop=(hdt == 3),
                    )
                    yield
                nc.vector.tensor_copy(osb[:, e2 * 512 : (e2 + 1) * 512], op)
                yield
            nc.sync.dma_start(out=out_d[t16sl, :], in_=osb)
            yield

        def chain(*gens):
            for g in gens:
                yield from g

        def drain(g):
            for _ in g:
                pass

        # ---- merged schedule ------------------------------------------
        # attention at query-window qt needs q/k/v only through window qt.
        # Only pair 0's projections run up front (getting the scarce exp
        # stream started early); the rest of window 0, window tt+1's
        # projections, and output-projection pieces are fed as PE filler
        # INSIDE the flattened attention windows, so the PE always has
        # independent matmuls while ACT streams the softmax exps.  Window
        # 0's filler leads with the remaining window-0 pieces: its own
        # pairs need their q/k ropes and v(t16 0-3).
        drain(proj_qkv_piece(0, 0))
        for tt in range(NTT):
            gens = []
            nfill = 0
            if tt == 0:
                for pc in ((0, 1), (0, 2), (0, 3), (1, 0), (1, 1), (1, 2), (1, 3)):
                    gens.append(proj_qkv_piece(*pc))
                    nfill += 27
            elif tt + 1 < NTT:
                for m in range(4):
                    gens.append(proj_qkv_piece(tt + 1, m))
                    nfill += 27
            if tt == 2:
                for t16 in range(4):
                    gens.append(proj_out_piece(t16))
                    nfill += 11
            elif tt == 3:
                for t16 in range(4, 12):
                    gens.append(proj_out_piece(t16))
                    nfill += 11
            filler = chain(*gens)
            window(tt, filler, nfill, last=(tt == NTT - 1))
            drain(filler)
        for m in range(4):
            drain(proj_out_piece(12 + m))


_NC_CACHE = None
LAST_RESULT = None


def _get_nc():
    global _NC_CACHE
    if _NC_CACHE is None:
        _NC_CACHE = _build_nc()
    return _NC_CACHE


def _rope_tables(start_pos):
    inv = 1.0 / (ROPE_BASE ** (np.arange(0, D, 2, dtype=np.float32) / D))
    t = np.arange(T, dtype=np.float32) + np.float32(start_pos)
    freqs = t[:, None] * inv[None, :]  # [T, 32]
    emb = np.concatenate([freqs, freqs], axis=-1)  # [T, 64]
    cos = np.cos(emb).T  # [64, T]
    sin = np.sin(emb).T
    sgn = np.where(np.arange(D) % 2 == 0, -1.0, 1.0).astype(np.float32)
    cosT = np.tile(cos, (2, 1))
    sinT = np.tile(sin * sgn[:, None], (2, 1))
    return cosT.astype(NPBF), sinT.astype(NPBF)


def kernel(x, Wq, Wk, Wv, Wp, start_pos):
    x = np.asarray(x, dtype=np.float32)
    Wq = np.asarray(Wq, dtype=np.float32)
    Wk = np.asarray(Wk, dtype=np.float32)
    Wv = np.asarray(Wv, dtype=np.float32)
    Wp = np.asarray(Wp, dtype=np.float32)
    cosT, sinT = _rope_tables(int(start_pos))

    nc = _get_nc()
    in_maps = []
    for c in range(8):
        b, g = divmod(c, 2)
        hs = slice(g * DL, (g + 1) * DL)
        in_maps.append(
            {
                "xT": np.ascontiguousarray(x[b].T).astype(NPBF),
                "wqT": np.ascontiguousarray(Wq[hs, :].T).astype(NPBF),
                "wkT": np.ascontiguousarray(Wk[hs, :].T).astype(NPBF),
                "wvT": np.ascontiguousarray(Wv[hs, :].T).astype(NPBF),
                "wpT": np.ascontiguousarray(Wp[:, hs].T).astype(NPBF),
                "cosT": cosT,
                "sinT": sinT,
            }
        )
    try:
        res = run_bass_kernel_spmd(nc, in_maps, core_ids=list(range(8)))
    except ModuleNotFoundError:
        # BASS_TRACE set but the axon NTFF hook module is unavailable in
        # this environment — rerun with tracing disabled.
        os.environ["BASS_NEVER_TRACE"] = "1"
        res = run_bass_kernel_spmd(nc, in_maps, core_ids=list(range(8)))
    global LAST_RESULT
    LAST_RESULT = res
    outs = [r["out"] for r in res.results]
    full = np.stack(
        [outs[2 * b] + outs[2 * b + 1] for b in range(B)], axis=0
    )
    return full.astype(np.float32)


if __name__ == "__main__":
    nc = _get_nc()
    print("built ok:", len(nc.m.functions[0].blocks if hasattr(nc.m.functions[0], 'blocks') else []), "blocks")

